# revision 18
# baseline (speedup 1.0000x reference)
"""Trainium2 Bass kernel for nn_BIAN_73057393705164 (GNN message passing).

Self-contained: host-side edge bucketing/sharding + 8-core SPMD Bass/Tile
kernel, executed via bass_utils.run_bass_kernel_spmd.

v3 design:
- node tables split into 4 "quarter" tables (<=25.6k rows each) so batched
  dma_gather (InstDMAGatherAnt, int16 indices) can fetch whole tile-chunks
  in one SWDGE op; per-quarter AllGathers pipeline with compute.
- edges bucketed by (node-block of the scatter key, quarter of the gather
  key); aggregation via one-hot bf16 matmuls into PSUM, accumulated across
  quarter passes in SBUF.
- time-encoding segment sums via host-precomputed per-block (node x ts)
  count matrices: te_sum = C_b @ T, no per-edge gather.
"""
import numpy as np
import concourse.bass as bass
import concourse.bacc as bacc
import concourse.mybir as mybir
import concourse.tile as tile
from concourse.bass_utils import run_bass_kernel_spmd

dt = mybir.dt
F32 = dt.float32
BF16 = dt.bfloat16
I16 = dt.int16
AL = mybir.AluOpType
RELU = mybir.ActivationFunctionType.Relu

N_NODES = 100000
N_EDGES = 800000
NCORES = 8
H = 64
ED = 32
ND = 128
TROWS = 1000
TCH = 125          # ts chunk rows (8 chunks of 125)
QR = 3200          # quarter size in local rows (last quarter 2900)
CH = 8             # tiles per gather chunk (1024 idxs = SWDGE ring cap)
XCH = 16           # blocks per x-input chunk

DMA_TYPES = (mybir.InstDMACopy, mybir.InstDMAGatherAnt, mybir.InstDMAScatterAddAnt)

S_SPLIT = False  # build one-hot S on gpsimd for odd tiles (else all on DVE)
AG_MODE = "quarter"  # "quarter": 4 per-quarter AllGathers/layer; "range": 1 full AllGather/layer

_fix_ctr = [0]


def fix_dma_waits(nc):
    """neuronxcc walrus accepts at most one sync wait per instruction; Tile
    can emit several on DMA-type instructions. Peel extra waits onto NoOps
    inserted just before, on the same engine."""
    nfix = 0
    for bb in nc.main_func.blocks:
        il = bb.instructions
        out = []
        changed = False
        for i in il:
            if i.sync_info is not None and len(i.sync_info.on_wait) > 1:
                for w in i.sync_info.on_wait:
                    _fix_ctr[0] += 1
                    nop = mybir.InstNoOp(name=f"I-FIXW-{_fix_ctr[0]}", ins=[], outs=[])
                    nop.engine = i.engine
                    nop.sync_info = mybir.SyncInfo(on_wait=[w], on_update=[])
                    out.append(nop)
                i.sync_info.on_wait = []
                changed = True
                nfix += 1
            out.append(i)
        if changed:
            bb.instructions = out
    return nfix


# ---------------------------------------------------------------- host prep

def quarter_sizes(SH):
    return [QR, QR, QR, SH - 3 * QR]


def q_of_local(rloc):
    return np.minimum(rloc // QR, 3)


def host_prep(inputs, N, E, ncores):
    SH = N // ncores
    NB = (SH + 127) // 128
    LB = SH - (NB - 1) * 128
    QS = quarter_sizes(SH)

    src = np.asarray(inputs["edge_index"][0]).astype(np.int64)
    dst = np.asarray(inputs["edge_index"][1]).astype(np.int64)
    ts = np.clip(np.asarray(inputs["timestamps"]).astype(np.int64), 0, TROWS - 1)
    x = np.asarray(inputs["x"], np.float32)
    ea = np.asarray(inputs["edge_attr"], np.float32)

    deg = np.bincount(dst, minlength=N).astype(np.float32) + 1.0
    outdeg = np.bincount(src, minlength=N).astype(np.float32)

    # global node id -> (quarter, index within quarter table)
    g_core = src * 0  # placeholder

    def qidx(nodes):
        if AG_MODE == "range":
            q = np.minimum(nodes >> 15, 3)
            off = nodes - (q << 15)
            return q, off.astype(np.int16)
        c = nodes // SH
        rloc = nodes % SH
        q = q_of_local(rloc)
        off = c * np.asarray(QS)[q] + (rloc - q * QR)
        return q, off.astype(np.int16)

    def bucket(key_node, gat_node):
        """Bucket edges by (block(key) within core, quarter(gat)).
        Returns per-core slot arrays + per-(b,q) tile counts (max over cores)
        + per-quarter stream tile counts."""
        per_core = []
        counts = np.zeros((ncores, NB, 4), np.int64)
        gq_all, _ = qidx(gat_node)
        for c in range(ncores):
            base = c * SH
            sel = (key_node >= base) & (key_node < base + SH)
            idxs = np.nonzero(sel)[0]
            blk = (key_node[idxs] - base) >> 7
            gq = gq_all[idxs]
            order = np.lexsort((idxs, gq, blk))
            idxs = idxs[order]
            blk = blk[order]
            gq = gq[order]
            cnt = np.zeros((NB, 4), np.int64)
            np.add.at(cnt, (blk, gq), 1)
            counts[c] = cnt
            per_core.append((idxs, cnt))
        tiles_bq = np.ceil(counts.max(0) / 128).astype(np.int64)  # [NB, 4]
        NTq = tiles_bq.sum(0)                                      # [4]
        # global tile order: q-major, then block, then tiles within cell
        slots_cores = []
        for c in range(ncores):
            idxs, cnt = per_core[c]
            # per (b, q) edge-id runs in sorted order
            run_off = np.zeros((NB, 4), np.int64)
            flat = cnt.ravel()
            ends = np.cumsum(flat).reshape(NB, 4)
            starts = ends - cnt
            slots = np.full(int(NTq.sum()) * 128, -1, np.int64)
            pos = 0
            for q in range(4):
                for b in range(NB):
                    nreal = int(cnt[b, q])
                    s0 = int(starts[b, q])
                    slots[pos:pos + nreal] = idxs[s0:s0 + nreal]
                    pos += int(tiles_bq[b, q]) * 128
            slots_cores.append(slots)
        return slots_cores, tiles_bq, NTq

    conv_slots, conv_tiles, conv_NTq = bucket(dst, src)
    fus_slots, fus_tiles, fus_NTq = bucket(src, dst)
    NT_C = int(conv_NTq.sum())
    NT_F = int(fus_NTq.sum())

    def wrap16(vals):
        """[Ntile*128] int16 -> [128, Ntile*8] with j -> [j%16, j//16],
        replicated down the 8 groups of 16 partitions."""
        n = vals.shape[0]
        blkw = vals.reshape(n // 16, 16).T  # [16, n//16]
        return np.tile(blkw, (8, 1)).copy()

    W = {k: np.asarray(inputs[k], np.float32) for k in
         ["Wn", "bn", "We", "be", "Temb", "Wt", "bt", "Wc1", "bc1",
          "Wc2", "bc2", "Wr", "br", "Wm1", "bm1", "Wm2", "bm2"]}
    iota_bf = np.broadcast_to(np.arange(128, dtype=np.float32),
                              (128, 128)).astype(ml_bf16())
    ident = np.eye(128, dtype=np.float32)
    ones_row = np.ones((1, 128), np.float32)
    ones_bf = np.ones((1, 128), ml_bf16())

    in_maps = []
    for c in range(ncores):
        base = c * SH
        pad_n = NB * 128 - SH
        deg_c = np.concatenate([deg[base:base + SH], np.ones(pad_n, np.float32)])
        out_c = np.concatenate([outdeg[base:base + SH], np.zeros(pad_n, np.float32)])
        deg_pb = deg_c.reshape(NB, 128).T.copy()
        out_pb = out_c.reshape(NB, 128).T.copy()
        xT = x[base:base + SH].T.copy()

        cs = conv_slots[c]
        valid = cs >= 0
        csrc = np.where(valid, src[np.maximum(cs, 0)], 0)
        _, coff = qidx(csrc)
        coff = np.where(valid, coff, 0).astype(np.int16)
        cdrel = np.where(valid, (dst[np.maximum(cs, 0)] - base) & 127,
                         -1).astype(np.float32)
        # split per quarter stream
        conv_i16 = []
        o = 0
        for q in range(4):
            nq = int(conv_NTq[q]) * 128
            conv_i16.append(wrap16(coff[o:o + nq]))
            o += nq
        conv_drel = cdrel.reshape(NT_C, 128).T.astype(ml_bf16()).copy()

        fs = fus_slots[c]
        fvalid = fs >= 0
        fdst = np.where(fvalid, dst[np.maximum(fs, 0)], 0)
        _, foff = qidx(fdst)
        foff = np.where(fvalid, foff, 0).astype(np.int16)
        fsrel = np.where(fvalid, (src[np.maximum(fs, 0)] - base) & 127,
                         -1).astype(np.float32)
        fus_i16 = []
        o = 0
        for q in range(4):
            nq = int(fus_NTq[q]) * 128
            fus_i16.append(wrap16(foff[o:o + nq]))
            o += nq
        fus_srel = fsrel.reshape(NT_F, 128).T.astype(ml_bf16()).copy()
        ea_rows = np.where(fvalid[:, None], ea[np.maximum(fs, 0)], 0.0)
        eaT = ea_rows.T.astype(ml_bf16()).copy()   # [32, NT_F*128]

        # count matrices: CT[p, (b*8+k)*128+n] = #edges(src=base+b*128+n, ts=k*125+p)
        sel = (src >= base) & (src < base + SH)
        s_loc = src[sel] - base
        t_loc = ts[sel]
        CT = np.zeros((TCH, NB * 8 * 128), np.float32)
        b_arr = s_loc >> 7
        n_arr = s_loc & 127
        k_arr = t_loc // TCH
        p_arr = t_loc % TCH
        np.add.at(CT, (p_arr, (b_arr * 8 + k_arr) * 128 + n_arr), 1.0)
        CT = CT.astype(ml_bf16())

        m = dict(
            xT=xT, deg=deg_pb, outdeg=out_pb,
            conv_drel=conv_drel, fus_srel=fus_srel,
            eaT=eaT, CT=CT,
            iota_bf=iota_bf, ident=ident, ones_row=ones_row, ones_bf=ones_bf,
            Wn=W["Wn"], Wc1=W["Wc1"], Wc2=W["Wc2"], Wr=W["Wr"],
            Wt=W["Wt"], wm1a=W["Wm1"][0:128], wm1b=W["Wm1"][128:256],
            Wm2=W["Wm2"], TembT=W["Temb"].T.copy(),
            We_bf=W["We"].astype(ml_bf16()), be_bf=W["be"][None, :].astype(ml_bf16()),
            bn=W["bn"][None, :], bc1=W["bc1"][None, :], bc2=W["bc2"][None, :],
            br=W["br"][None, :], bt=W["bt"][None, :],
            bm1=W["bm1"][None, :], bm2=W["bm2"][None, :],
        )
        for q in range(4):
            m[f"conv_i16_{q}"] = conv_i16[q]
            m[f"fus_i16_{q}"] = fus_i16[q]
        in_maps.append(m)

    meta = dict(N=N, E=E, SH=SH, NB=NB, LB=LB, NT_C=NT_C, NT_F=NT_F,
                conv_tiles=conv_tiles.tolist(), fus_tiles=fus_tiles.tolist(),
                conv_NTq=conv_NTq.tolist(), fus_NTq=fus_NTq.tolist(),
                QS=QS, ncores=ncores, ag_mode=AG_MODE)
    return in_maps, meta


def ml_bf16():
    import ml_dtypes
    return ml_dtypes.bfloat16


# ---------------------------------------------------------------- builder

def build(meta, repeat=1):
    N, SH, NB, LB = meta["N"], meta["SH"], meta["NB"], meta["LB"]
    NT_C, NT_F = meta["NT_C"], meta["NT_F"]
    conv_tiles = meta["conv_tiles"]   # [NB][4]
    fus_tiles = meta["fus_tiles"]
    conv_NTq, fus_NTq = meta["conv_NTq"], meta["fus_NTq"]
    QS = meta["QS"]
    ncores = meta["ncores"]
    QBLK = [25, 25, 25, NB - 75]      # blocks per quarter

    nc = bacc.Bacc(None)
    P = nc.declare_dram_parameter
    xT_in = P("xT", [ND, SH], F32, isOutput=False)
    deg_in = P("deg", [128, NB], F32, isOutput=False)
    outdeg_in = P("outdeg", [128, NB], F32, isOutput=False)
    cdrel_in = P("conv_drel", [128, NT_C], BF16, isOutput=False)
    fsrel_in = P("fus_srel", [128, NT_F], BF16, isOutput=False)
    ci16_in = [P(f"conv_i16_{q}", [128, conv_NTq[q] * 8], I16, isOutput=False)
               for q in range(4)]
    fi16_in = [P(f"fus_i16_{q}", [128, fus_NTq[q] * 8], I16, isOutput=False)
               for q in range(4)]
    eaT_in = P("eaT", [ED, NT_F * 128], BF16, isOutput=False)
    CT_in = P("CT", [TCH, NB * 8 * 128], BF16, isOutput=False)
    iota_in = P("iota_bf", [128, 128], BF16, isOutput=False)
    ident_in = P("ident", [128, 128], F32, isOutput=False)
    ones_in = P("ones_row", [1, 128], F32, isOutput=False)
    onesbf_in = P("ones_bf", [1, 128], BF16, isOutput=False)
    w_ins = {}
    for nm, shp, dty in [
            ("Wn", [ND, H], F32), ("Wc1", [H, H], F32), ("Wc2", [H, H], F32),
            ("Wr", [H, H], F32), ("Wt", [H, H], F32),
            ("wm1a", [128, H], F32), ("wm1b", [128, H], F32), ("Wm2", [H, 2], F32),
            ("TembT", [H, TROWS], F32),
            ("We_bf", [ED, H], BF16), ("be_bf", [1, H], BF16),
            ("bn", [1, H], F32), ("bc1", [1, H], F32), ("bc2", [1, H], F32),
            ("br", [1, H], F32), ("bt", [1, H], F32),
            ("bm1", [1, H], F32), ("bm2", [1, 2], F32)]:
        w_ins[nm] = P(nm, shp, dty, isOutput=False)
    out_ext = P("out", [SH, 2], F32, isOutput=True)

    # tile maps: per (q, b): (global tile offset within stream q, ntiles)
    def tmap(tiles):
        offs = [[None] * NB for _ in range(4)]
        for q in range(4):
            o = 0
            for b in range(NB):
                offs[q][b] = (o, int(tiles[b][q]))
                o += int(tiles[b][q])
        return offs
    conv_map = tmap(conv_tiles)
    fus_map = tmap(fus_tiles)

    def rows(b):
        return LB if b == NB - 1 else 128

    with tile.TileContext(nc, num_cores=ncores) as tc:
        with (
            tc.tile_pool(name="cst", bufs=1) as cst,
            tc.tile_pool(name="shard", bufs=1) as shp_,
            tc.tile_pool(name="sb", bufs=3) as sb,
            tc.tile_pool(name="spool", bufs=4) as spool,
            tc.tile_pool(name="gst", bufs=3) as gst,
            tc.tile_pool(name="gsb", bufs=3) as gsb,
            tc.tile_pool(name="gidx", bufs=3) as gidx,
            tc.tile_pool(name="gea", bufs=2) as gea,
            tc.tile_pool(name="gct", bufs=2) as gct,
            tc.tile_pool(name="psA", bufs=2, space="PSUM") as psA,
            tc.tile_pool(name="psE", bufs=2, space="PSUM") as psE,
            tc.tile_pool(name="psB", bufs=2, space="PSUM") as psB,
            tc.tile_pool(name="psC", bufs=2, space="PSUM") as psC,
            tc.tile_pool(name="dram", bufs=1, space="DRAM") as drp,
        ):
            def ld(pool, handle, shape, dtype=F32, tag=None):
                t = pool.tile(shape, dtype, tag=tag or handle.name)
                nc.sync.dma_start(out=t[:], in_=handle[:])
                return t

            iota_bf = ld(cst, iota_in, [128, 128], BF16)
            ident = ld(cst, ident_in, [128, 128])
            ones_row = ld(cst, ones_in, [1, 128])
            ones_bf = ld(cst, onesbf_in, [1, 128], BF16)
            wt = {nm: ld(cst, w_ins[nm], list(w_ins[nm].shape),
                         w_ins[nm].dtype) for nm in w_ins}
            cdrel = ld(cst, cdrel_in, [128, NT_C], BF16)
            fsrel = ld(cst, fsrel_in, [128, NT_F], BF16)
            deg_t = ld(cst, deg_in, [128, NB])
            outdeg_t = ld(cst, outdeg_in, [128, NB])

            dinv = cst.tile([128, NB], F32, tag="dinv")
            nc.scalar.sqrt(dinv[:], deg_t[:])
            nc.vector.reciprocal(dinv[:], dinv[:])
            cinv = cst.tile([128, NB], F32, tag="cinv")
            nc.vector.tensor_scalar_max(cinv[:], outdeg_t[:], 1.0)
            nc.vector.reciprocal(cinv[:], cinv[:])

            # SBUF shards / accumulators
            x_enc = shp_.tile([128, NB * H], F32, tag="x_enc")
            accA = shp_.tile([128, NB * H], F32, tag="accA")
            accB = shp_.tile([128, NB * H], F32, tag="accB")
            accC = shp_.tile([128, NB * H], F32, tag="accC")
            xw = shp_.tile([128, 25 * H], F32, tag="xw")

            # DRAM internals
            ag_mode = meta.get("ag_mode", "quarter")
            agin = [drp.tile([SH, H], F32, name=f"agin{l}", tag=f"agin{l}")
                    for l in range(3)]
            tabs = [None, None, None]
            ftabs = [None, None, None]
            RS = [32768, 32768, 32768, N - 3 * 32768]

            def make_tabs(rep):
                for l in range(3):
                    if ag_mode == "range":
                        ftabs[l] = drp.tile([N, H], F32, addr_space="Shared",
                                            name=f"ftab{l}r{rep}", tag=f"ftab{l}r{rep}")
                        tabs[l] = [ftabs[l][q * 32768:q * 32768 + RS[q], :]
                                   for q in range(4)]
                    else:
                        tabs[l] = [drp.tile([ncores * QS[q], H], F32,
                                            addr_space="Shared",
                                            name=f"tab{l}q{q}r{rep}",
                                            tag=f"tab{l}q{q}r{rep}")
                                   for q in range(4)]
            rgroups = [list(range(ncores))]

            def rank1(ps, bias, ones=None, start=False, stop=False):
                o = ones if ones is not None else ones_row
                nc.tensor.matmul(out=ps, lhsT=o[:, :ps.partition_size()],
                                 rhs=bias, start=start, stop=stop)

            # T chunks (bf16, in SBUF)
            T_sb = []
            for k in range(8):
                o = k * TCH
                pT = psB.tile([128, H], F32, tag="pnode")
                nc.tensor.matmul(out=pT[:TCH, :], lhsT=wt["TembT"][:, o:o + TCH],
                                 rhs=wt["Wt"][:], start=True, stop=False)
                rank1(pT[:TCH, :], wt["bt"][:], stop=True)
                tt = cst.tile([TCH, H], BF16, tag=f"T_sb{k}")
                nc.scalar.activation(tt[:], pT[:TCH, :], RELU)
                T_sb.append(tt)

            # quarter boundaries in blocks: q covers blocks [qb0[q], qb0[q+1])
            qb0 = [0, 25, 50, 75, NB]

            def write_xw(layer, q):
                """DMA xw quarter buffer -> agin[layer] rows."""
                r0 = qb0[q] * 128
                nrows = QS[q]
                full_b = (nrows // 128)
                rem = nrows - full_b * 128
                av = agin[layer][r0:r0 + full_b * 128, :].rearrange(
                    "(b p) h -> p b h", p=128)
                nc.sync.dma_start(out=av, in_=xw[:, :full_b * H].rearrange(
                    "p (b h) -> p b h", h=H))
                if rem:
                    nc.sync.dma_start(
                        out=agin[layer][r0 + full_b * 128:r0 + nrows, :],
                        in_=xw[:rem, full_b * H:full_b * H + H])

            def ag_only(layer, q):
                r0 = qb0[q] * 128
                nrows = QS[q]
                if ag_mode == "range":
                    if q == 3:
                        nc.gpsimd.collective_compute(
                            "AllGather", AL.bypass, replica_groups=rgroups,
                            ins=[agin[layer][:].opt()], outs=[ftabs[layer][:].opt()])
                else:
                    nc.gpsimd.collective_compute(
                        "AllGather", AL.bypass, replica_groups=rgroups,
                        ins=[agin[layer][r0:r0 + nrows, :].opt()],
                        outs=[tabs[layer][q].opt()])

            def emit_ag(layer, q):
                write_xw(layer, q)
                ag_only(layer, q)

            # ---------------- phase 0: x_enc + table0 quarters
            xchunks = {}

            def xfetch(b0):
                n = min(XCH * 128, SH - b0 * 128)
                xs = gst.tile([128, XCH * 128], F32, tag="xchunk")
                nc.sync.dma_start(out=xs[:, :n],
                                  in_=xT_in[:, b0 * 128:b0 * 128 + n])
                return xs

            xchunks[0] = xfetch(0)
            for b in range(NB):
                n = rows(b)
                ci = b // XCH
                if b % XCH == 0 and (ci + 1) * XCH < NB:
                    xchunks[ci + 1] = xfetch((ci + 1) * XCH)
                xs = xchunks[ci][:, (b % XCH) * 128:(b % XCH) * 128 + 128]
                pN = psB.tile([128, H], F32, tag="pnode")
                nc.tensor.matmul(out=pN[:n, :], lhsT=xs[:, :n], rhs=wt["Wn"][:],
                                 start=True, stop=False)
                rank1(pN[:n, :], wt["bn"][:], stop=True)
                cols = slice(b * H, b * H + H)
                if n < 128:
                    nc.vector.memset(x_enc[:, cols], 0.0)
                nc.scalar.activation(x_enc[:, cols][:n, :], pN[:n, :], RELU)
                q = min(b // 25, 3)
                wcols = slice((b - qb0[q]) * H, (b - qb0[q]) * H + H)
                nc.vector.tensor_scalar_mul(xw[:, wcols], x_enc[:, cols],
                                            dinv[:, b:b + 1])
                if b + 1 == qb0[q + 1]:
                    write_xw(0, q)

            # ---------------- gather machinery
            def gfetch(table, i16, t0, NTq):
                tn = min(CH, NTq - t0)
                ic = gidx.tile([128, CH * 8], I16, tag="ic")
                nc.sync.dma_start(out=ic[:, :tn * 8], in_=i16[:, t0 * 8:(t0 + tn) * 8])
                g = gst.tile([128, CH * H], F32, tag="gchunk")
                gv = g[:, :tn * H].rearrange("p (t h) -> p t h", h=H)
                tap = table if isinstance(table, bass.AP) else table[:]
                nc.gpsimd.dma_gather(
                    out_ap=gv, in_ap=tap, idxs_ap=ic[:, :tn * 8],
                    num_idxs=tn * 128, num_idxs_reg=tn * 128, elem_size=H)
                gb = gsb.tile([128, CH * H], BF16, tag="gb")
                nc.scalar.activation(gb[:, :tn * H], g[:, :tn * H],
                                     mybir.ActivationFunctionType.Copy)
                return gb

            def build_S(rel, t, engine):
                S = spool.tile([128, 128], BF16, tag="S")
                engine.tensor_tensor(
                    out=S[:], in0=rel[:, t:t + 1].to_broadcast([128, 128]),
                    in1=iota_bf[:], op=AL.is_equal)
                return S

            # ---------------- conv layers
            def conv(layer):
                acc = accA
                nc.vector.memset(acc[:], 0.0)
                table = tabs[layer - 1]
                for q in range(4):
                    NTq = conv_NTq[q]
                    if NTq == 0:
                        continue
                    chunks = {0: gfetch(table[q], ci16_in[q], 0, NTq)}
                    for b in range(NB):
                        off, nt = conv_map[q][b]
                        if nt == 0:
                            continue
                        pA = psA.tile([128, H], F32, tag="agg")
                        for j in range(nt):
                            t = off + j
                            ci = t // CH
                            if t % CH == 0 and (ci + 1) * CH < NTq:
                                chunks[ci + 1] = gfetch(table[q], ci16_in[q],
                                                        (ci + 1) * CH, NTq)
                            gidx_t = qoff_c[q] + t
                            eng = nc.gpsimd if (S_SPLIT and t % 2) else nc.vector
                            S = build_S(cdrel, gidx_t, eng)
                            G = chunks[ci][:, (t % CH) * H:(t % CH) * H + H]
                            nc.tensor.matmul(out=pA[:], lhsT=S[:], rhs=G,
                                             start=(j == 0), stop=(j == nt - 1))
                        cols = slice(b * H, b * H + H)
                        nc.vector.tensor_add(out=acc[:, cols], in0=acc[:, cols],
                                             in1=pA[:])
                # drain sweep
                for b in range(NB):
                    n = rows(b)
                    cols = slice(b * H, b * H + H)
                    sp = sb.tile([128, H], F32, tag="sp")
                    if layer == 1:
                        nc.vector.tensor_scalar_mul(sp[:], x_enc[:, cols],
                                                    dinv[:, b:b + 1])
                    else:
                        if n < 128:
                            nc.vector.memset(sp[:], 0.0)
                        nc.sync.dma_start(out=sp[:n, :],
                                          in_=agin[1][b * 128:b * 128 + n, :])
                    tD = sb.tile([128, H], F32, tag="tD")
                    nc.vector.tensor_add(out=tD[:], in0=acc[:, cols], in1=sp[:])
                    nc.vector.tensor_scalar_mul(tD[:], tD[:], dinv[:, b:b + 1])
                    pTr = psC.tile([128, 128], F32, tag="tr")
                    nc.tensor.transpose(out=pTr[:H, :], in_=tD[:], identity=ident[:])
                    tT = sb.tile([H, 128], F32, tag="tT")
                    nc.vector.tensor_copy(out=tT[:], in_=pTr[:H, :])
                    pX = psB.tile([128, H], F32, tag="pnode")
                    if layer == 1:
                        nc.tensor.matmul(out=pX[:], lhsT=tT[:], rhs=wt["Wc1"][:],
                                         start=True, stop=False)
                        rank1(pX[:], wt["bc1"][:], stop=True)
                    else:
                        nc.tensor.matmul(out=pX[:], lhsT=tT[:], rhs=wt["Wc2"][:],
                                         start=True, stop=False)
                        pTr2 = psC.tile([128, 128], F32, tag="tr")
                        nc.tensor.transpose(out=pTr2[:H, :], in_=x_enc[:, cols],
                                            identity=ident[:])
                        xeT = sb.tile([H, 128], F32, tag="xeT")
                        nc.vector.tensor_copy(out=xeT[:], in_=pTr2[:H, :])
                        nc.tensor.matmul(out=pX[:], lhsT=xeT[:], rhs=wt["Wr"][:],
                                         start=False, stop=False)
                        rank1(pX[:], wt["bc2"][:])
                        rank1(pX[:], wt["br"][:], stop=True)
                    q = min(b // 25, 3)
                    wcols = slice((b - qb0[q]) * H, (b - qb0[q]) * H + H)
                    if layer == 1:
                        # write x1 * dinv into xw (table1 rows) and agin[0]
                        y = sb.tile([128, H], F32, tag="ydr")
                        nc.scalar.activation(y[:], pX[:], RELU)
                        nc.vector.tensor_scalar_mul(xw[:, wcols], y[:],
                                                    dinv[:, b:b + 1])
                    else:
                        nc.scalar.activation(xw[:, wcols], pX[:], RELU)
                    if b + 1 == qb0[q + 1]:
                        emit_ag(layer, q)

            # offsets of quarter streams within the global (drel/srel) order
            qoff_c = [0] * 4
            for q in range(1, 4):
                qoff_c[q] = qoff_c[q - 1] + conv_NTq[q - 1]
            qoff_f = [0] * 4
            for q in range(1, 4):
                qoff_f[q] = qoff_f[q - 1] + fus_NTq[q - 1]

            # ---------------- fusion
            def fusion():
                # te pass: accA[b] = sum_e T[ts[e]]  (C_b @ T)
                nc.vector.memset(accA[:], 0.0)
                GCT = 2
                ctchunks = {}

                def ctfetch(b0):
                    nb = min(GCT, NB - b0)
                    t_ = gct.tile([TCH, GCT * 8 * 128], BF16, tag="ct")
                    nc.sync.dma_start(out=t_[:, :nb * 8 * 128],
                                      in_=CT_in[:, b0 * 8 * 128:(b0 + nb) * 8 * 128])
                    return t_

                ctchunks[0] = ctfetch(0)
                for b in range(NB):
                    ci = b // GCT
                    if b % GCT == 0 and (ci + 1) * GCT < NB:
                        ctchunks[ci + 1] = ctfetch((ci + 1) * GCT)
                    ct = ctchunks[ci]
                    boff = (b % GCT) * 8 * 128
                    pT = psE.tile([128, H], F32, tag="te")
                    for k in range(8):
                        nc.tensor.matmul(
                            out=pT[:], lhsT=ct[:, boff + k * 128:boff + (k + 1) * 128],
                            rhs=T_sb[k][:], start=(k == 0), stop=(k == 7))
                    cols = slice(b * H, b * H + H)
                    nc.vector.tensor_copy(out=accA[:, cols], in_=pT[:])

                # x2[dst] + edge-enc passes
                nc.vector.memset(accB[:], 0.0)
                nc.vector.memset(accC[:], 0.0)
                for q in range(4):
                    NTq = fus_NTq[q]
                    if NTq == 0:
                        continue
                    chunks = {0: gfetch(tabs[2][q], fi16_in[q], 0, NTq)}
                    eachunks = {}

                    def eafetch(t0):
                        tn = min(CH, NTq - t0)
                        t_ = gea.tile([ED, CH * 128], BF16, tag="ea")
                        g0 = (qoff_f[q] + t0) * 128
                        nc.sync.dma_start(out=t_[:, :tn * 128],
                                          in_=eaT_in[:, g0:g0 + tn * 128])
                        return t_

                    eachunks[0] = eafetch(0)
                    for b in range(NB):
                        off, nt = fus_map[q][b]
                        if nt == 0:
                            continue
                        pX = psA.tile([128, H], F32, tag="agg")
                        pE = psE.tile([128, H], F32, tag="te")
                        for j in range(nt):
                            t = off + j
                            ci = t // CH
                            if t % CH == 0 and (ci + 1) * CH < NTq:
                                chunks[ci + 1] = gfetch(tabs[2][q], fi16_in[q],
                                                        (ci + 1) * CH, NTq)
                                eachunks[ci + 1] = eafetch((ci + 1) * CH)
                            gt = qoff_f[q] + t
                            eng = nc.gpsimd if (S_SPLIT and t % 2) else nc.vector
                            pEE = psB.tile([128, H], F32, tag="pnode")
                            nc.tensor.matmul(
                                out=pEE[:],
                                lhsT=eachunks[ci][:, (t % CH) * 128:(t % CH + 1) * 128],
                                rhs=wt["We_bf"][:], start=True, stop=False)
                            rank1(pEE[:], wt["be_bf"][:], ones=ones_bf, stop=True)
                            ee = sb.tile([128, H], BF16, tag="ee")
                            nc.scalar.activation(ee[:], pEE[:], RELU)
                            S = build_S(fsrel, gt, eng)
                            G = chunks[ci][:, (t % CH) * H:(t % CH) * H + H]
                            nc.tensor.matmul(out=pX[:], lhsT=S[:], rhs=G,
                                             start=(j == 0), stop=(j == nt - 1))
                            nc.tensor.matmul(out=pE[:], lhsT=S[:], rhs=ee[:],
                                             start=(j == 0), stop=(j == nt - 1))
                        cols = slice(b * H, b * H + H)
                        nc.vector.tensor_add(out=accC[:, cols], in0=accC[:, cols],
                                             in1=pX[:])
                        nc.vector.tensor_add(out=accB[:, cols], in0=accB[:, cols],
                                             in1=pE[:])

                # final sweep
                for b in range(NB):
                    n = rows(b)
                    cols = slice(b * H, b * H + H)
                    nf = sb.tile([128, 4 * H], F32, tag="nf")
                    x2o = sb.tile([128, H], F32, tag="x2o")
                    if n < 128:
                        nc.vector.memset(x2o[:], 0.0)
                    nc.sync.dma_start(out=x2o[:n, :],
                                      in_=agin[2][b * 128:b * 128 + n, :])
                    nc.vector.tensor_copy(out=nf[:, 0:H], in_=x2o[:])
                    nc.vector.tensor_scalar_mul(nf[:, H:2 * H], accC[:, cols],
                                                cinv[:, b:b + 1])
                    nc.vector.tensor_scalar_mul(nf[:, 2 * H:3 * H], accB[:, cols],
                                                cinv[:, b:b + 1])
                    nc.vector.tensor_scalar_mul(nf[:, 3 * H:4 * H], accA[:, cols],
                                                cinv[:, b:b + 1])
                    mask = sb.tile([128, 1], F32, tag="mask")
                    nc.vector.tensor_scalar(out=mask[:], in0=outdeg_t[:, b:b + 1],
                                            scalar1=0.5, scalar2=None, op0=AL.is_lt)
                    for g in range(4):
                        gsl = slice(g * H, (g + 1) * H)
                        dd = sb.tile([128, H], F32, tag="dd")
                        nc.vector.tensor_sub(out=dd[:], in0=x_enc[:, cols],
                                             in1=nf[:, gsl])
                        nc.vector.scalar_tensor_tensor(
                            out=nf[:, gsl], in0=dd[:], scalar=mask[:],
                            in1=nf[:, gsl], op0=AL.mult, op1=AL.add)
                    pH = psB.tile([128, H], F32, tag="pnode")
                    for half, w in [(0, "wm1a"), (1, "wm1b")]:
                        pTr = psC.tile([128, 128], F32, tag="tr")
                        nc.tensor.transpose(out=pTr[:],
                                            in_=nf[:, half * 128:(half + 1) * 128],
                                            identity=ident[:])
                        nfT = sb.tile([128, 128], F32, tag="nfT")
                        nc.vector.tensor_copy(out=nfT[:], in_=pTr[:])
                        nc.tensor.matmul(out=pH[:], lhsT=nfT[:], rhs=wt[w][:],
                                         start=(half == 0), stop=False)
                    rank1(pH[:], wt["bm1"][:], stop=True)
                    h1 = sb.tile([128, H], F32, tag="h1")
                    nc.scalar.activation(h1[:], pH[:], RELU)
                    pTr = psC.tile([128, 128], F32, tag="tr")
                    nc.tensor.transpose(out=pTr[:H, :], in_=h1[:], identity=ident[:])
                    h1T = sb.tile([H, 128], F32, tag="h1T")
                    nc.vector.tensor_copy(out=h1T[:], in_=pTr[:H, :])
                    pO = psB.tile([128, H], F32, tag="pnode")
                    nc.tensor.matmul(out=pO[:, :2], lhsT=h1T[:], rhs=wt["Wm2"][:],
                                     start=True, stop=False)
                    rank1(pO[:, :2], wt["bm2"][:], stop=True)
                    ot = sb.tile([128, 2], F32, tag="ot")
                    nc.vector.tensor_copy(out=ot[:], in_=pO[:, :2])
                    nc.sync.dma_start(out=out_ext[b * 128:b * 128 + n, :],
                                      in_=ot[:n, :])

            for _rep in range(repeat):
                make_tabs(_rep)
                for q in range(4):
                    ag_only(0, q)
                conv(1)
                conv(2)
                fusion()

    nc.finalize()
    return nc


# ---------------------------------------------------------------- entry

def kernel(**inputs) -> np.ndarray:
    in_maps, meta = host_prep(inputs, N_NODES, N_EDGES, NCORES)
    nc = build(meta)
    fix_dma_waits(nc)
    res = run_bass_kernel_spmd(nc, in_maps, core_ids=list(range(NCORES)))
    out = np.concatenate([np.asarray(res.results[c]["out"]) for c in range(NCORES)],
                         axis=0)
    return out.astype(np.float32)
